# revision 19
# baseline (speedup 1.0000x reference)
"""LIF (leaky integrate-and-fire) forward recurrence on 8 Trainium2 NeuronCores.

Input  x: (T=16, B=128, N=16384) float32, time-major.
    m[t] = tau * v[t-1] + x[t]
    y[t] = (m[t] >= v_th)            spike, as 0.0/1.0
    v[t] = m[t] * (1 - y[t])         hard reset

Sharding: N split 8 ways (2048 per core); the recurrence is per-neuron
independent so the cores never communicate.  The host re-lays each shard
as (B, T, N) so a multi-timestep DMA chunk reads long contiguous runs
per SBUF partition row.

The fp32 recurrence pins two 2-source ops per step to the DVE (1x mode,
(151+W)/0.96 ns) - the critical resource.  To shrink it, the 2048
columns are split into two slices with different engine pipelines:

  slice A (cols 0:1280, m-space, DVE-owned):
    m_A = (v_A * tau) + x       stt on DVE
    sig = Sign(1 - m_A)         ScalarE -> int8 {+1,0,-1}; the OUTPUT
                                (host: spike = sig <= 0)
    v_A = (m_A < 1) * m_A       stt on DVE

  slice C (cols 1280:2048, scaled space s[t] = 2^t * m[t], so the
  tau-multiply vanishes and the add is PLAIN - runnable on gpsimd):
    s_C[t] = w_C[t-1] + xs[t]   tensor_tensor add on GPSIMD
                                (xs = 2^t * x, exact host prescale)
    sig    = Sign(1 - 2^-t s)   ScalarE, scale immediate = -2^-t
    w_C[t] = (sig > 0) * s_C    stt on DVE (int8 predicate, one s read)

Every power-of-2 scale is exact in f32, the Sign compare and {0,1}
multiplies are exact, so the result stays bit-identical to the f32
reference.  Supporting arrangement: inputs on the Sync HWDGE ring,
outputs on the Scalar ring (rings are FIFO; mixing directions
head-of-line blocks), 3 rotating input-chunk buffers with a fine early
ramp, v/w[T-1] skipped, final step stored in column halves.
(PE identity-matmuls, gpsimd tensor_scalar, and CCE accum-DMA all
measured slower on this toolchain.)
"""

import numpy as np

import concourse.bass as bass
import concourse.mybir as mybir
from concourse.bass_utils import run_bass_kernel_spmd
from concourse.mybir import AluOpType
from concourse.tile import TileContext

T, B, N = 16, 128, 16384
NCORES = 8
NSH = N // NCORES  # 2048 neurons per core
TAU = 0.5
V_TH = 1.0

CA = 1280  # slice A columns [0:CA) on the DVE pipeline
# slice C columns [CA:NSH) on the gpsimd-add pipeline

IN_CHUNKS = [1, 1, 1, 1, 2, 2, 4, 4]
OUT_CHUNKS = [4, 4, 4, 2, 1, 1]

_cached_nc = None


def _split_multiwaits(nc):
    """Walrus codegen in this toolchain supports only ONE sync-wait per
    instruction (single wait slot in the EVENTS field); Tile sometimes
    attaches two or more.  Move the extra waits onto same-engine NoOps
    inserted right before - the sequencer executes in program order, so
    semantics are unchanged."""
    multi_ok = (mybir.InstEventSemaphore, mybir.InstNoOp)
    for f in nc.m.functions:
        for b in f.blocks:
            new_insts = []
            for inst in b.instructions:
                si = inst.sync_info
                if (
                    not isinstance(inst, multi_ok)
                    and si is not None
                    and len(si.on_wait) > 1
                ):
                    waits = list(si.on_wait)
                    for j, w in enumerate(waits[:-1]):
                        new_insts.append(
                            mybir.InstNoOp(
                                name=f"{inst.name}_presync{j}",
                                engine=inst.engine,
                                sync_info=mybir.SyncInfo(on_wait=[w], on_update=[]),
                            )
                        )
                    inst.sync_info = mybir.SyncInfo(
                        on_wait=[waits[-1]], on_update=list(si.on_update)
                    )
                new_insts.append(inst)
            b.instructions = new_insts


def _build():
    nc = bass.Bass(trn_type="TRN2")
    x = nc.dram_tensor("x", [B, T, NSH], mybir.dt.float32, kind="ExternalInput")
    y = nc.dram_tensor("y", [B, T, NSH], mybir.dt.int8, kind="ExternalOutput")

    with TileContext(nc) as tc:
        with (
            tc.tile_pool(name="state", bufs=1) as state_pool,
            tc.tile_pool(name="xin", bufs=3) as xin_pool,
            tc.tile_pool(name="yout", bufs=2) as yout_pool,
            tc.tile_pool(name="work", bufs=3) as work_pool,
        ):
            v_a = state_pool.tile([B, CA], mybir.dt.float32)
            w_c = [
                state_pool.tile([B, NSH - CA], mybir.dt.float32, name=f"wc{p}")
                for p in range(2)
            ]
            s_c = [
                state_pool.tile([B, NSH - CA], mybir.dt.float32, name=f"sc{p}")
                for p in range(2)
            ]

            xt_tiles = {}
            t0 = 0
            for ci, w in enumerate(IN_CHUNKS):
                xt = xin_pool.tile(
                    [B, 4, NSH], mybir.dt.float32, tag="xt", name=f"xt{ci}"
                )
                # all input loads on the Sync HWDGE ring (outputs ride the
                # Scalar ring: FIFO per ring, so mixing input and output on
                # one ring would head-of-line block it)
                nc.sync.dma_start(out=xt[:, :w, :], in_=x[:, t0 : t0 + w, :])
                for k in range(w):
                    xt_tiles[t0 + k] = xt[:, k, :]
                t0 += w

            out_t0 = 0
            oc = 0
            yt = None
            for t in range(T):
                p = t % 2
                if yt is None:
                    yt = yout_pool.tile(
                        [B, 4, NSH], mybir.dt.int8, tag="yt", name=f"yt{oc}"
                    )
                xk = xt_tiles[t]
                # ---- slice A: m-space on DVE ----
                if t == 0:
                    m_a = xk[:, :CA]  # v==0 -> m = x[0]
                else:
                    mt = work_pool.tile([B, CA], mybir.dt.float32, tag="m", name="mt")
                    nc.vector.scalar_tensor_tensor(
                        mt[:], v_a[:], TAU, xk[:, :CA], AluOpType.mult, AluOpType.add
                    )
                    m_a = mt[:]
                # ---- slice C: s-space; s_C[t] tile ----
                if t == 0:
                    s_cur = xk[:, CA:]  # s[0] = xs[0] = x[0]
                else:
                    s_cur = s_c[p][:]
                yo = t - out_t0
                last = t == T - 1
                # sig: one Sign per slice; this is the output chunk AND the
                # reset predicate.  (On the last step the stores fire per
                # slice right after their sig to shorten the drain.)
                nc.scalar.activation(
                    yt[:, yo, :CA], m_a,
                    mybir.ActivationFunctionType.Sign,
                    bias=V_TH, scale=-1.0,
                )
                if last:
                    nc.scalar.dma_start(
                        out=y[:, t : t + 1, :CA], in_=yt[:, yo : yo + 1, :CA]
                    )
                nc.scalar.activation(
                    yt[:, yo, CA:], s_cur,
                    mybir.ActivationFunctionType.Sign,
                    bias=V_TH, scale=-(2.0 ** -t),
                )
                if last:
                    nc.scalar.dma_start(
                        out=y[:, t : t + 1, CA:], in_=yt[:, yo : yo + 1, CA:]
                    )
                if not last:
                    # v_A = (m_A < 1) * m_A
                    nc.vector.scalar_tensor_tensor(
                        v_a[:], m_a, V_TH, m_a, AluOpType.is_lt, AluOpType.mult
                    )
                    # w_C = (sig > 0) * s_C   [sig>0 <=> m<1]
                    nc.vector.scalar_tensor_tensor(
                        w_c[p][:], yt[:, yo, CA:], 0.0, s_cur,
                        AluOpType.is_gt, AluOpType.mult,
                    )
                    # s_C[t+1] = w_C[t] + xs[t+1] on gpsimd
                    nc.gpsimd.tensor_tensor(
                        s_c[1 - p][:], w_c[p][:], xt_tiles[t + 1][:, CA:],
                        AluOpType.add,
                    )
                if t - out_t0 + 1 == OUT_CHUNKS[oc]:
                    w = OUT_CHUNKS[oc]
                    if not last:  # last step already stored by slices
                        nc.scalar.dma_start(
                            out=y[:, out_t0 : out_t0 + w, :], in_=yt[:, :w, :]
                        )
                    out_t0 += w
                    oc += 1
                    yt = None
    _split_multiwaits(nc)
    return nc


def kernel(x: np.ndarray) -> np.ndarray:
    global _cached_nc
    if _cached_nc is None:
        _cached_nc = _build()
    nc = _cached_nc

    x = np.ascontiguousarray(x, dtype=np.float32)
    assert x.shape == (T, B, N)
    # (T, B, N) -> per-core (B, T, NSH) shards, timestep-contiguous rows
    xbt = np.ascontiguousarray(x.transpose(1, 0, 2))
    # slice C runs in scaled space: xs[t] = 2^t * x[t] (exact in f32)
    scale = (2.0 ** np.arange(T, dtype=np.float32))[None, :, None]
    in_maps = []
    for k in range(NCORES):
        sh = np.ascontiguousarray(xbt[:, :, k * NSH : (k + 1) * NSH])
        sh[:, :, CA:] *= scale
        in_maps.append({"x": sh})
    res = run_bass_kernel_spmd(nc, in_maps, core_ids=list(range(NCORES)))
    global _last_exec_ns
    if res.exec_time_ns is not None:
        _last_exec_ns = res.exec_time_ns
    # per-core int8 sign (B, T, NSH): sig = Sign(1 - m-scaled), spike <=> sig <= 0
    out = np.concatenate([r["y"] for r in res.results], axis=2)
    return (
        np.ascontiguousarray(out.transpose(1, 0, 2)) <= 0
    ).astype(np.float32)


_last_exec_ns = None


# revision 20
# speedup vs baseline: 1.1395x; 1.1395x over previous
"""LIF (leaky integrate-and-fire) forward recurrence on 8 Trainium2 NeuronCores.

Input  x: (T=16, B=128, N=16384) float32, time-major.
    m[t] = tau * v[t-1] + x[t]
    y[t] = (m[t] >= v_th)            spike, as 0.0/1.0
    v[t] = m[t] * (1 - y[t])         hard reset

Sharding: N split 8 ways (2048 per core); the recurrence is per-neuron
independent so the cores never communicate.  The host re-lays each shard
as (B, T, N) so a multi-timestep DMA chunk reads/writes long contiguous
runs per SBUF partition row.

Per core per timestep the work is a [128 x 2048] f32 tile:
    m   = (v * tau) + x[t]       scalar_tensor_tensor on DVE
    sig = Sign(1 - m)            ScalarE -> int8 {+1,0,-1}; the OUTPUT
                                 (host: spike = sig <= 0) - one ACT op
                                 per step instead of two
    v'  = (m < 1) * m            scalar_tensor_tensor on DVE

The recurrence makes the DVE stt pair the critical path (~2.29 us per
fp32 1x op, back-to-back in program order); sig reads only m so the
chain never waits on the Scalar engine.  Everything else is arranged
around keeping that stream fed from t=0:
  - inputs on the Sync HWDGE ring, outputs on the Scalar ring (rings
    are FIFO; mixing directions head-of-line blocks),
  - 3 rotating input-chunk buffers with a fine early ramp so chunk
    k lands before the DVE needs step k,
  - v[T-1] skipped (nothing consumes it), final sig/store split into
    column halves to shorten the drain.
All ops are exact in f32, so the result is bit-identical to the f32
reference.  (PE identity-matmul, gpsimd elementwise, and DMA-accum
variants were all measured slower on this toolchain: fp32 matmul runs
2 LDWEIGHTS+MATMUL passes per instruction, gpsimd tensor ops run at
3-15 ns/elem, and CCE accum-DMA tops out at ~200 GB/s.)
"""

import numpy as np

import concourse.bass as bass
import concourse.mybir as mybir
from concourse.bass_utils import run_bass_kernel_spmd
from concourse.mybir import AluOpType
from concourse.tile import TileContext

T, B, N = 16, 128, 16384
NCORES = 8
NSH = N // NCORES  # 2048 neurons per core
TAU = 0.5
V_TH = 1.0

IN_CHUNKS = [1, 1, 1, 1, 2, 2, 4, 4]
OUT_CHUNKS = [4, 4, 4, 2, 1, 1]

_cached_nc = None


def _split_multiwaits(nc):
    """Walrus codegen in this toolchain supports only ONE sync-wait per
    instruction (single wait slot in the EVENTS field); Tile sometimes
    attaches two or more.  Move the extra waits onto same-engine NoOps
    inserted right before - the sequencer executes in program order, so
    semantics are unchanged."""
    multi_ok = (mybir.InstEventSemaphore, mybir.InstNoOp)
    for f in nc.m.functions:
        for b in f.blocks:
            new_insts = []
            for inst in b.instructions:
                si = inst.sync_info
                if (
                    not isinstance(inst, multi_ok)
                    and si is not None
                    and len(si.on_wait) > 1
                ):
                    waits = list(si.on_wait)
                    for j, w in enumerate(waits[:-1]):
                        new_insts.append(
                            mybir.InstNoOp(
                                name=f"{inst.name}_presync{j}",
                                engine=inst.engine,
                                sync_info=mybir.SyncInfo(on_wait=[w], on_update=[]),
                            )
                        )
                    inst.sync_info = mybir.SyncInfo(
                        on_wait=[waits[-1]], on_update=list(si.on_update)
                    )
                new_insts.append(inst)
            b.instructions = new_insts


def _build():
    nc = bass.Bass(trn_type="TRN2")
    x = nc.dram_tensor("x", [B, T, NSH], mybir.dt.float32, kind="ExternalInput")
    y = nc.dram_tensor("y", [B, T, NSH], mybir.dt.int8, kind="ExternalOutput")

    with TileContext(nc) as tc:
        with (
            tc.tile_pool(name="state", bufs=1) as state_pool,
            tc.tile_pool(name="xin", bufs=4) as xin_pool,
            tc.tile_pool(name="yout", bufs=2) as yout_pool,
            tc.tile_pool(name="work", bufs=3) as work_pool,
        ):
            v = state_pool.tile([B, NSH], mybir.dt.float32)

            xt_tiles = {}
            t0 = 0
            for ci, w in enumerate(IN_CHUNKS):
                xt = xin_pool.tile(
                    [B, 4, NSH], mybir.dt.float32, tag="xt", name=f"xt{ci}"
                )
                # all input loads on the Sync HWDGE ring (outputs ride the
                # Scalar ring: FIFO per ring, so mixing input and output on
                # one ring would head-of-line block it)
                nc.sync.dma_start(out=xt[:, :w, :], in_=x[:, t0 : t0 + w, :])
                for k in range(w):
                    xt_tiles[t0 + k] = xt[:, k, :]
                t0 += w

            out_t0 = 0
            oc = 0
            yt = None
            for t in range(T):
                if yt is None:
                    yt = yout_pool.tile(
                        [B, 4, NSH], mybir.dt.int8, tag="yt", name=f"yt{oc}"
                    )
                xk = xt_tiles[t]
                if t == 0:
                    m = xk  # v==0 -> m = x[0]
                else:
                    mt = work_pool.tile(
                        [B, NSH], mybir.dt.float32, tag="m", name="mt"
                    )
                    # m = v * tau + x[t]
                    nc.vector.scalar_tensor_tensor(
                        mt[:], v[:], TAU, xk, AluOpType.mult, AluOpType.add
                    )
                    m = mt[:]
                # sig = Sign(1 - m) -> int8; host: spike = (sig <= 0).
                # The last step is split into column halves so the final
                # output DMA can start as early as possible.
                if t == T - 1:
                    for h0 in (0, NSH // 2):
                        hs = slice(h0, h0 + NSH // 2)
                        nc.scalar.activation(
                            yt[:, t - out_t0, hs], m[:, hs],
                            mybir.ActivationFunctionType.Sign,
                            bias=V_TH, scale=-1.0,
                        )
                        nc.scalar.dma_start(
                            out=y[:, t : t + 1, hs], in_=yt[:, t - out_t0 : t - out_t0 + 1, hs]
                        )
                else:
                    nc.scalar.activation(
                        yt[:, t - out_t0, :], m,
                        mybir.ActivationFunctionType.Sign,
                        bias=V_TH, scale=-1.0,
                    )
                # v = (m < v_th) * m   (hard reset; off the ACT path).
                # Skipped for the last step - nothing consumes v[T-1].
                if t < T - 1:
                    nc.vector.scalar_tensor_tensor(
                        v[:], m, V_TH, m, AluOpType.is_lt, AluOpType.mult
                    )
                if t - out_t0 + 1 == OUT_CHUNKS[oc]:
                    w = OUT_CHUNKS[oc]
                    if t < T - 1:  # last step already stored by halves
                        nc.scalar.dma_start(
                            out=y[:, out_t0 : out_t0 + w, :], in_=yt[:, :w, :]
                        )
                    out_t0 += w
                    oc += 1
                    yt = None
    _split_multiwaits(nc)
    return nc


def kernel(x: np.ndarray) -> np.ndarray:
    global _cached_nc
    if _cached_nc is None:
        _cached_nc = _build()
    nc = _cached_nc

    x = np.ascontiguousarray(x, dtype=np.float32)
    assert x.shape == (T, B, N)
    # (T, B, N) -> per-core (B, T, NSH) shards, timestep-contiguous rows
    xbt = np.ascontiguousarray(x.transpose(1, 0, 2))
    in_maps = [
        {"x": np.ascontiguousarray(xbt[:, :, k * NSH : (k + 1) * NSH])}
        for k in range(NCORES)
    ]
    res = run_bass_kernel_spmd(nc, in_maps, core_ids=list(range(NCORES)))
    global _last_exec_ns
    if res.exec_time_ns is not None:
        _last_exec_ns = res.exec_time_ns
    # per-core int8 sign (B, T, NSH): sig = Sign(1-m), spike <=> sig <= 0
    out = np.concatenate([r["y"] for r in res.results], axis=2)
    return (
        np.ascontiguousarray(out.transpose(1, 0, 2)) <= 0
    ).astype(np.float32)


_last_exec_ns = None
